# revision 7
# baseline (speedup 1.0000x reference)
"""Trainium2 Bass kernel for nn_BinLinear: out = x @ where(clip(w,-1,1) >= 0, 1, -1).

Since clipping to [-1, 1] preserves sign, the binarized weight is exactly
where(w >= 0, +1, -1), so the kernel computes out = x @ sign01(w) as a dense
matmul on the TensorEngine.

Distribution: x is sharded row-wise across the 8 NeuronCores (1024 rows each);
the binarized w streams through each core once.  Each core computes an
independent [1024, 4096] output shard; the host concatenates shards (no
device collectives needed).

Precision/layout variants (BL_VARIANT env var):
  f16x1  - x cast to fp16, single matmul pass.            ~2e-4 rel err
  f16x2  - x split hi/lo fp16, two accumulating passes.   ~1e-7 rel err
  bf16x2 - x split hi/lo bf16, two accumulating passes.   ~3e-6 rel err
  f32r   - x kept fp32, matmul in float32r (tf32-like).   ~1e-4 rel err
"""

import os
import sys

for _p in ("/opt/trn_rl_repo", "/root/.axon_site/_ro/trn_rl_repo"):
    if os.path.isdir(_p) and _p not in sys.path:
        sys.path.append(_p)

import numpy as np
import ml_dtypes

import concourse.bacc as bacc
import concourse.mybir as mybir
from concourse.tile import TileContext
from concourse.bass_utils import run_bass_kernel_spmd

P = 128
NCORES = 8
B_FULL, K_DIM, N_FULL = 8192, 4096, 4096
M_CORE = B_FULL // NCORES  # 1024

VARIANT = os.environ.get("BL_VARIANT", "f16x1")

_NC_CACHE = {}


def _build_xstat(n_passes, mmdt, panel, repeat=1):
    """Variant A: stationary = x^T tiles, moving = w panels, out natural [M, N].

    Per-core inputs: xt_hi [K, M_CORE] (+ xt_lo), wb [K, N_FULL] in mmdt.
    Output: out [M_CORE, N_FULL] fp32.
    repeat>1 wraps the compute in a hardware loop (for benchmarking).
    """
    import contextlib

    dt = mybir.dt
    KC = K_DIM // P
    MT = M_CORE // P
    NPAN = N_FULL // panel

    nc = bacc.Bacc("TRN2")
    xt_parts = [
        nc.dram_tensor(
            "xt_hi" if i == 0 else "xt_lo", [K_DIM, M_CORE], mmdt, kind="ExternalInput"
        )
        for i in range(n_passes)
    ]
    # w pre-packed on host so each (panel, kc-pair) DMA reads 2*panel
    # contiguous elements per partition (2-4KB lines instead of 0.5-1KB).
    wb_d = nc.dram_tensor(
        "wb", [KC // 2, P, NPAN, 2 * panel], mmdt, kind="ExternalInput"
    )
    out_d = nc.dram_tensor("out", [M_CORE, N_FULL], dt.float32, kind="ExternalOutput")

    with TileContext(nc) as tc:
        with (
            tc.tile_pool(name="xpool", bufs=1) as xpool,
            tc.tile_pool(name="wpool", bufs=2) as wpool,
            tc.tile_pool(name="opool", bufs=3) as opool,
            tc.tile_pool(name="pspool", bufs=8, space="PSUM") as pspool,
        ):
            xs = []
            xt_rs = []
            for i, xt_d in enumerate(xt_parts):
                xt_rs.append(xt_d.rearrange("(kc p) m -> p kc m", p=P))
                xs.append(xpool.tile([P, KC, M_CORE], mmdt, name=f"x{i}"))

            loop_cm = (
                tc.For_i(
                    0,
                    repeat,
                    1,
                    hint_engines=(
                        mybir.EngineType.PE,
                        mybir.EngineType.SP,
                        mybir.EngineType.DVE,
                    ),
                    name="rep",
                )
                if repeat > 1
                else contextlib.nullcontext()
            )
            def evict(ps, mt, ip):
                ot = opool.tile([P, panel], dt.float32, name="ot")
                nc.vector.tensor_copy(ot[:], ps[:])
                nc.sync.dma_start(
                    out=out_d[mt * P : (mt + 1) * P, ip * panel : (ip + 1) * panel],
                    in_=ot[:],
                )

            # x slab loads interleaved with panel-0 w loads so panel-0
            # compute (kc-outer, all 8 psum banks) tracks slab arrival.
            with loop_cm:
                # panel 0: kc-outer / mt-inner
                wt0 = wpool.tile([P, KC, panel], mmdt, name="wt")
                for kc2 in range(KC // 2):
                    nc.sync.dma_start(
                        out=wt0[:, 2 * kc2 : 2 * kc2 + 2, :], in_=wb_d[kc2, :, 0, :]
                    )
                    for i in range(n_passes):
                        nc.sync.dma_start(
                            out=xs[i][:, 2 * kc2, :], in_=xt_rs[i][:, 2 * kc2, :]
                        )
                        nc.sync.dma_start(
                            out=xs[i][:, 2 * kc2 + 1, :],
                            in_=xt_rs[i][:, 2 * kc2 + 1, :],
                        )
                pss = [
                    pspool.tile([P, panel], dt.float32, name="ps") for _ in range(MT)
                ]
                for kc in range(KC):
                    for mt in range(MT):
                        for ipass in range(n_passes):
                            nc.tensor.matmul(
                                pss[mt][:],
                                lhsT=xs[ipass][:, kc, mt * P : (mt + 1) * P],
                                rhs=wt0[:, kc, :],
                                start=(kc == 0 and ipass == 0),
                                stop=(kc == KC - 1 and ipass == n_passes - 1),
                                skip_group_check=True,
                            )
                for mt in range(MT):
                    evict(pss[mt], mt, 0)

                # panels 1..NPAN-1: mt-outer (single psum in flight)
                for ip in range(1, NPAN):
                    wt = wpool.tile([P, KC, panel], mmdt, name="wt")
                    for kc2 in range(KC // 2):
                        nc.sync.dma_start(
                            out=wt[:, 2 * kc2 : 2 * kc2 + 2, :],
                            in_=wb_d[kc2, :, ip, :],
                        )
                    for mt in range(MT):
                        ps = pspool.tile([P, panel], dt.float32, name="ps")
                        for kc in range(KC):
                            for ipass in range(n_passes):
                                nc.tensor.matmul(
                                    ps[:],
                                    lhsT=xs[ipass][:, kc, mt * P : (mt + 1) * P],
                                    rhs=wt[:, kc, :],
                                    start=(kc == 0 and ipass == 0),
                                    stop=(kc == KC - 1 and ipass == n_passes - 1),
                                )
                        evict(ps, mt, ip)
    nc.compile()
    return nc


def _build_hybrid(G, repeat=1):
    """Hybrid fp8-DoubleRow + fp16 xstat kernel.

    K is split: first K8 = G*256 contraction rows go through fp8e4m3
    DoubleRow matmuls (2 k per PE cell per cycle, ~1.9x the fp16 rate);
    the remaining K16 = 4096 - K8 rows go through fp16 matmuls.  Both
    accumulate into the same PSUM bank per (mt, panel) stream.

    Per-core inputs (pre-packed on host):
      x8  [G, P, 2*M_CORE]          e4m3   x^T pair-tiles (k, k+128 within group)
      x16 [KC16, P, M_CORE]         fp16   x^T tiles for the fp16 part
      w8  [G, P, NPAN, 2*panel]     e4m3   binarized w panels, pair-packed
      w16 [KC16//2, P, NPAN, 2*panel] fp16 binarized w panels (kc-pair packed)
    Output: out [M_CORE, N_FULL] fp32.
    """
    import contextlib

    dt = mybir.dt
    panel = 512
    NPAN = N_FULL // panel
    MT = M_CORE // P
    K8 = G * 256
    KC16 = (K_DIM - K8) // P
    assert KC16 % 2 == 0

    nc = bacc.Bacc("TRN2")
    x8_d = nc.dram_tensor("x8", [G, P, 2 * M_CORE], dt.float8e4, kind="ExternalInput")
    x16_d = nc.dram_tensor("x16", [KC16, P, M_CORE], dt.float16, kind="ExternalInput")
    w8_d = nc.dram_tensor(
        "w8", [G, P, NPAN, 2 * panel], dt.float8e4, kind="ExternalInput"
    )
    w16_d = nc.dram_tensor(
        "w16", [KC16 // 2, P, NPAN, 2 * panel], dt.float16, kind="ExternalInput"
    )
    out_d = nc.dram_tensor("out", [M_CORE, N_FULL], dt.float32, kind="ExternalOutput")

    with TileContext(nc) as tc:
        with (
            tc.tile_pool(name="xpool", bufs=2) as xpool,
            tc.tile_pool(name="w8pool", bufs=2) as w8pool,
            tc.tile_pool(name="w16pool", bufs=2) as w16pool,
            tc.tile_pool(name="opool", bufs=3) as opool,
            tc.tile_pool(name="pspool", bufs=8, space="PSUM") as pspool,
        ):
            x8_r = x8_d.rearrange("g p (two m) -> g p two m", two=2)

            loop_cm = (
                tc.For_i(
                    0,
                    repeat,
                    1,
                    hint_engines=(
                        mybir.EngineType.PE,
                        mybir.EngineType.SP,
                        mybir.EngineType.DVE,
                    ),
                    name="rep",
                )
                if repeat > 1
                else contextlib.nullcontext()
            )

            def evict(ps, mt, ip):
                ot = opool.tile([P, panel], dt.float32, name="ot")
                nc.vector.tensor_copy(ot[:], ps[:])
                nc.sync.dma_start(
                    out=out_d[mt * P : (mt + 1) * P, ip * panel : (ip + 1) * panel],
                    in_=ot[:],
                )

            def mm_stream(ps, x8t, x16t, w8t, w16t, mt, g_lo=0, g_hi=None,
                          kc_lo=0, kc_hi=None, first=True, last=True):
                g_hi = G if g_hi is None else g_hi
                kc_hi = KC16 if kc_hi is None else kc_hi
                for g in range(g_lo, g_hi):
                    nc.tensor.matmul(
                        ps[:],
                        lhsT=x8t[:, g, :, mt * P : (mt + 1) * P],
                        rhs=w8t[:, g, :, :],
                        start=(g == g_lo and first),
                        stop=False,
                        perf_mode=mybir.MatmulPerfMode.DoubleRow,
                        skip_group_check=True,
                    )
                for kc in range(kc_lo, kc_hi):
                    nc.tensor.matmul(
                        ps[:],
                        lhsT=x16t[:, kc, mt * P : (mt + 1) * P],
                        rhs=w16t[:, kc, :],
                        start=(g_hi == g_lo and kc == kc_lo and first),
                        stop=(kc == kc_hi - 1 and last),
                        skip_group_check=True,
                    )

            w8_r = w8_d.rearrange("g p np (two n) -> g p np two n", two=2)
            w16_r = w16_d.rearrange("kc2 p np (two n) -> kc2 p np two n", two=2)

            with loop_cm:
                x8t = xpool.tile([P, G, 2, M_CORE], dt.float8e4, name="x8t")
                x16t = xpool.tile([P, KC16, M_CORE], dt.float16, name="x16t")
                # ---- panel 0: two mt-outer k-sections; section A's slabs DMA
                # first so its compute starts while section B's slabs stream in.
                w8t0 = w8pool.tile([P, G, 2, panel], dt.float8e4, name="w8t")
                w16t0 = w16pool.tile([P, KC16, panel], dt.float16, name="w16t")
                GA = (G + 1) // 2
                KA2 = (KC16 // 2) // 2
                KA = 2 * KA2
                for g in range(GA):
                    nc.sync.dma_start(out=w8t0[:, g, :, :], in_=w8_r[g, :, 0])
                    nc.sync.dma_start(out=x8t[:, g, :, :], in_=x8_r[g])
                for kc2 in range(KA2):
                    nc.sync.dma_start(
                        out=w16t0[:, 2 * kc2 : 2 * kc2 + 2, :], in_=w16_r[kc2, :, 0]
                    )
                    nc.sync.dma_start(out=x16t[:, 2 * kc2, :], in_=x16_d[2 * kc2])
                    nc.sync.dma_start(
                        out=x16t[:, 2 * kc2 + 1, :], in_=x16_d[2 * kc2 + 1]
                    )
                for g in range(GA, G):
                    nc.sync.dma_start(out=w8t0[:, g, :, :], in_=w8_r[g, :, 0])
                    nc.sync.dma_start(out=x8t[:, g, :, :], in_=x8_r[g])
                for kc2 in range(KA2, KC16 // 2):
                    nc.sync.dma_start(
                        out=w16t0[:, 2 * kc2 : 2 * kc2 + 2, :], in_=w16_r[kc2, :, 0]
                    )
                    nc.sync.dma_start(out=x16t[:, 2 * kc2, :], in_=x16_d[2 * kc2])
                    nc.sync.dma_start(
                        out=x16t[:, 2 * kc2 + 1, :], in_=x16_d[2 * kc2 + 1]
                    )
                pss = [
                    pspool.tile([P, panel], dt.float32, name="ps") for _ in range(MT)
                ]
                for mt in range(MT):
                    mm_stream(pss[mt], x8t, x16t, w8t0, w16t0, mt,
                              g_hi=GA, kc_hi=KA, last=False)
                for mt in range(MT):
                    mm_stream(pss[mt], x8t, x16t, w8t0, w16t0, mt,
                              g_lo=GA, kc_lo=KA, first=False)
                    evict(pss[mt], mt, 0)

                # ---- panels 1..NPAN-1: mt-outer, double-buffered w tiles
                for ip in range(1, NPAN):
                    w8t = w8pool.tile([P, G, 2, panel], dt.float8e4, name="w8t")
                    w16t = w16pool.tile([P, KC16, panel], dt.float16, name="w16t")
                    for g in range(G):
                        nc.sync.dma_start(out=w8t[:, g, :, :], in_=w8_r[g, :, ip])
                    for kc2 in range(KC16 // 2):
                        nc.sync.dma_start(
                            out=w16t[:, 2 * kc2 : 2 * kc2 + 2, :],
                            in_=w16_r[kc2, :, ip],
                        )
                    for mt in range(MT):
                        ps = pspool.tile([P, panel], dt.float32, name="ps")
                        mm_stream(ps, x8t, x16t, w8t, w16t, mt)
                        evict(ps, mt, ip)
    nc.compile()
    return nc


def _build_wstat_f32r(bchunk=512, repeat=1):
    """Variant B (f32r): stationary = w column tiles, moving = resident x^T,
    out transposed [N_FULL, M_CORE].

    Per-core inputs: xt [K, M_CORE] fp32, wb [K, N_FULL] fp32.
    Output: out_t [N_FULL, M_CORE] fp32 (host transposes back).
    """
    dt = mybir.dt
    KC = K_DIM // P
    JT = N_FULL // P  # 32 stationary column tiles of w
    BC = M_CORE // bchunk  # moving chunks of x

    nc = bacc.Bacc("TRN2")
    xt_d = nc.dram_tensor("xt", [K_DIM, M_CORE], dt.float32r, kind="ExternalInput")
    wb_d = nc.dram_tensor("wb", [K_DIM, N_FULL], dt.float32r, kind="ExternalInput")
    out_d = nc.dram_tensor("out", [N_FULL, M_CORE], dt.float32, kind="ExternalOutput")

    with TileContext(nc) as tc:
        with (
            tc.tile_pool(name="xpool", bufs=1) as xpool,
            tc.tile_pool(name="wpool", bufs=3) as wpool,
            tc.tile_pool(name="opool", bufs=3) as opool,
            tc.tile_pool(name="pspool", bufs=4, space="PSUM") as pspool,
        ):
            xt_r = xt_d.rearrange("(kc p) m -> p kc m", p=P)
            xtile = xpool.tile([P, KC, M_CORE], dt.float32r, name="xres")
            for half in range(2):  # split load so compute starts early
                h = KC // 2
                nc.sync.dma_start(
                    out=xtile[:, half * h : (half + 1) * h, :],
                    in_=xt_r[:, half * h : (half + 1) * h, :],
                )

            wb_r = wb_d.rearrange("(kc p) n -> p kc n", p=P)

            import contextlib

            loop_cm = (
                tc.For_i(
                    0,
                    repeat,
                    1,
                    hint_engines=(
                        mybir.EngineType.PE,
                        mybir.EngineType.SP,
                        mybir.EngineType.DVE,
                    ),
                    name="rep",
                )
                if repeat > 1
                else contextlib.nullcontext()
            )
            with loop_cm:
                _body_wstat(nc, tc, wpool, opool, pspool, wb_r, xtile, out_d, KC, JT, BC, bchunk)
    nc.compile()
    return nc


def _body_wstat(nc, tc, wpool, opool, pspool, wb_r, xtile, out_d, KC, JT, BC, bchunk):
    dt = mybir.dt
    for jt in range(JT):
        wt = wpool.tile([P, KC, P], dt.float32r, name="wt")
        for kc in range(KC):
            nc.sync.dma_start(
                out=wt[:, kc, :],
                in_=wb_r[:, kc, jt * P : (jt + 1) * P],
            )
        pss = []
        for bc in range(BC):
            ps = pspool.tile([P, bchunk], dt.float32, name="ps")
            pss.append(ps)
        for kc in range(KC):
            for bc in range(BC):
                nc.tensor.matmul(
                    pss[bc][:],
                    lhsT=wt[:, kc, :],
                    rhs=xtile[:, kc, bc * bchunk : (bc + 1) * bchunk],
                    start=(kc == 0),
                    stop=(kc == KC - 1),
                    skip_group_check=True,
                )
        for bc in range(BC):
            ot = opool.tile([P, bchunk], dt.float32, name="ot")
            nc.vector.tensor_copy(ot[:], pss[bc][:])
            nc.sync.dma_start(
                out=out_d[jt * P : (jt + 1) * P, bc * bchunk : (bc + 1) * bchunk],
                in_=ot[:],
            )


def get_nc(variant=None, repeat=1):
    variant = variant or VARIANT
    key = (variant, repeat)
    if key not in _NC_CACHE:
        if variant == "f16x1":
            _NC_CACHE[key] = _build_xstat(1, mybir.dt.float16, 512, repeat)
        elif variant == "f16x2":
            _NC_CACHE[key] = _build_xstat(2, mybir.dt.float16, 256, repeat)
        elif variant == "bf16x2":
            _NC_CACHE[key] = _build_xstat(2, mybir.dt.bfloat16, 256, repeat)
        elif variant == "f32r":
            _NC_CACHE[key] = _build_wstat_f32r(512, repeat)
        elif variant.startswith("hyb"):
            _NC_CACHE[key] = _build_hybrid(int(variant[3:]), repeat)
        else:
            raise ValueError(f"unknown variant {variant}")
    return _NC_CACHE[key]


def _prep_hybrid(x, w, G):
    """Host prep for the hybrid variant: binarize w, split K, quantize/pack."""
    panel = 512
    NPAN = N_FULL // panel
    K8 = G * 256
    KC16 = (K_DIM - K8) // P
    f8 = ml_dtypes.float8_e4m3

    x = np.ascontiguousarray(x, dtype=np.float32)
    wb = np.where(np.asarray(w) >= 0, np.float32(1.0), np.float32(-1.0))
    xt = np.ascontiguousarray(x.T)  # [K, B]

    # x fp8 part: [K8, B] -> [G, 2, P, B] -> [G, P, 2, B]
    x8 = xt[:K8].astype(f8).reshape(G, 2, P, B_FULL).transpose(0, 2, 1, 3)
    x8 = np.ascontiguousarray(x8)  # [G, P, 2, B]
    # x fp16 part: [K16, B] -> [KC16, P, B]
    x16 = np.ascontiguousarray(xt[K8:].astype(np.float16).reshape(KC16, P, B_FULL))

    # w fp8 part: [K8, N] -> (G, 2, P, N) -> (G, P, 2, NPAN, panel) -> [G, P, NPAN, 2*panel]
    w8 = (
        wb[:K8]
        .astype(f8)
        .reshape(G, 2, P, NPAN, panel)
        .transpose(0, 2, 3, 1, 4)
        .reshape(G, P, NPAN, 2 * panel)
    )
    w8 = np.ascontiguousarray(w8)
    # w fp16 part: baseline kc-pair packing
    w16 = np.ascontiguousarray(
        wb[K8:]
        .astype(np.float16)
        .reshape(KC16 // 2, 2, P, NPAN, panel)
        .transpose(0, 2, 3, 1, 4)
        .reshape(KC16 // 2, P, NPAN, 2 * panel)
    )

    in_maps = []
    for c in range(NCORES):
        sl = slice(c * M_CORE, (c + 1) * M_CORE)
        in_maps.append(
            {
                "x8": np.ascontiguousarray(x8[:, :, :, sl]).reshape(G, P, 2 * M_CORE),
                "x16": np.ascontiguousarray(x16[:, :, sl]),
                "w8": w8,
                "w16": w16,
            }
        )
    return in_maps


def prep_in_maps(x, w, variant=None):
    """Host-side prep: binarize w, transpose/cast/split x, build per-core maps."""
    variant = variant or VARIANT
    if variant.startswith("hyb"):
        return _prep_hybrid(x, w, int(variant[3:]))
    x = np.ascontiguousarray(x, dtype=np.float32)
    wb = np.where(np.asarray(w) >= 0, np.float32(1.0), np.float32(-1.0))
    xt = np.ascontiguousarray(x.T)  # [K, B]

    if variant == "f32r":
        in_maps = []
        for c in range(NCORES):
            sl = slice(c * M_CORE, (c + 1) * M_CORE)
            in_maps.append({"xt": np.ascontiguousarray(xt[:, sl]), "wb": wb})
        return in_maps

    npdt = {"f16x1": np.float16, "f16x2": np.float16, "bf16x2": ml_dtypes.bfloat16}[
        variant
    ]
    n_passes = 1 if variant == "f16x1" else 2
    panel = 512 if variant == "f16x1" else 256
    KC, NPAN = K_DIM // P, N_FULL // panel
    xt_hi = xt.astype(npdt)
    wb16 = np.ascontiguousarray(
        wb.astype(npdt)
        .reshape(KC // 2, 2, P, NPAN, panel)
        .transpose(0, 2, 3, 1, 4)
        .reshape(KC // 2, P, NPAN, 2 * panel)
    )
    if n_passes == 2:
        xt_lo = (xt - xt_hi.astype(np.float32)).astype(npdt)

    in_maps = []
    for c in range(NCORES):
        sl = slice(c * M_CORE, (c + 1) * M_CORE)
        m = {"xt_hi": np.ascontiguousarray(xt_hi[:, sl]), "wb": wb16}
        if n_passes == 2:
            m["xt_lo"] = np.ascontiguousarray(xt_lo[:, sl])
        in_maps.append(m)
    return in_maps


def gather_out(results, variant=None):
    variant = variant or VARIANT
    if variant == "f32r":
        return np.concatenate(
            [np.asarray(results[c]["out"]).T for c in range(NCORES)], axis=0
        )
    return np.concatenate([np.asarray(results[c]["out"]) for c in range(NCORES)], axis=0)


def kernel(x, w):
    """Full inputs in, full output out.  x [8192, 4096] f32, w [4096, 4096] f32."""
    assert x.shape == (B_FULL, K_DIM) and w.shape == (K_DIM, N_FULL)
    nc = get_nc()
    in_maps = prep_in_maps(x, w)
    res = run_bass_kernel_spmd(nc, in_maps, core_ids=list(range(NCORES)))
    out = gather_out(res.results)
    return np.ascontiguousarray(out, dtype=np.float32)



# revision 11
# speedup vs baseline: 1.0239x; 1.0239x over previous
"""Trainium2 Bass kernel for nn_BinLinear: out = x @ where(clip(w,-1,1) >= 0, 1, -1).

Since clipping to [-1, 1] preserves sign, the binarized weight is exactly
where(w >= 0, +1, -1), so the kernel computes out = x @ sign01(w) as a dense
matmul on the TensorEngine.

Distribution: x is sharded row-wise across the 8 NeuronCores (1024 rows each);
the binarized w streams through each core once.  Each core computes an
independent [1024, 4096] output shard; the host concatenates shards (no
device collectives needed).

Precision/layout variants (BL_VARIANT env var):
  f16x1  - x cast to fp16, single matmul pass.            ~2e-4 rel err
  f16x2  - x split hi/lo fp16, two accumulating passes.   ~1e-7 rel err
  bf16x2 - x split hi/lo bf16, two accumulating passes.   ~3e-6 rel err
  f32r   - x kept fp32, matmul in float32r (tf32-like).   ~1e-4 rel err
"""

import os
import sys

for _p in ("/opt/trn_rl_repo", "/root/.axon_site/_ro/trn_rl_repo"):
    if os.path.isdir(_p) and _p not in sys.path:
        sys.path.append(_p)

import numpy as np
import ml_dtypes

import concourse.bacc as bacc
import concourse.mybir as mybir
from concourse.tile import TileContext
from concourse.bass_utils import run_bass_kernel_spmd

P = 128
NCORES = 8
B_FULL, K_DIM, N_FULL = 8192, 4096, 4096
M_CORE = B_FULL // NCORES  # 1024

VARIANT = os.environ.get("BL_VARIANT", "f16x1")

_NC_CACHE = {}


def _build_xstat(n_passes, mmdt, panel, repeat=1):
    """Variant A: stationary = x^T tiles, moving = w panels, out natural [M, N].

    Per-core inputs: xt_hi [K, M_CORE] (+ xt_lo), wb [K, N_FULL] in mmdt.
    Output: out [M_CORE, N_FULL] fp32.
    repeat>1 wraps the compute in a hardware loop (for benchmarking).
    """
    import contextlib

    dt = mybir.dt
    KC = K_DIM // P
    MT = M_CORE // P
    NPAN = N_FULL // panel

    nc = bacc.Bacc("TRN2")
    xt_parts = [
        nc.dram_tensor(
            "xt_hi" if i == 0 else "xt_lo", [K_DIM, M_CORE], mmdt, kind="ExternalInput"
        )
        for i in range(n_passes)
    ]
    # w pre-packed on host so each (panel, kc-pair) DMA reads 2*panel
    # contiguous elements per partition (2-4KB lines instead of 0.5-1KB).
    wb_d = nc.dram_tensor(
        "wb", [KC // 2, P, NPAN, 2 * panel], mmdt, kind="ExternalInput"
    )
    out_d = nc.dram_tensor("out", [M_CORE, N_FULL], dt.float32, kind="ExternalOutput")

    with TileContext(nc) as tc:
        with (
            tc.tile_pool(name="xpool", bufs=1) as xpool,
            tc.tile_pool(name="wpool", bufs=2) as wpool,
            tc.tile_pool(name="opool", bufs=3) as opool,
            tc.tile_pool(name="pspool", bufs=8, space="PSUM") as pspool,
        ):
            xs = []
            xt_rs = []
            for i, xt_d in enumerate(xt_parts):
                xt_rs.append(xt_d.rearrange("(kc p) m -> p kc m", p=P))
                xs.append(xpool.tile([P, KC, M_CORE], mmdt, name=f"x{i}"))

            loop_cm = (
                tc.For_i(
                    0,
                    repeat,
                    1,
                    hint_engines=(
                        mybir.EngineType.PE,
                        mybir.EngineType.SP,
                        mybir.EngineType.DVE,
                    ),
                    name="rep",
                )
                if repeat > 1
                else contextlib.nullcontext()
            )
            def evict(ps, mt, ip):
                ot = opool.tile([P, panel], dt.float32, name="ot")
                nc.vector.tensor_copy(ot[:], ps[:])
                nc.sync.dma_start(
                    out=out_d[mt * P : (mt + 1) * P, ip * panel : (ip + 1) * panel],
                    in_=ot[:],
                )

            # x slab loads interleaved with panel-0 w loads so panel-0
            # compute (kc-outer, all 8 psum banks) tracks slab arrival.
            with loop_cm:
                # panel 0: kc-outer / mt-inner
                wt0 = wpool.tile([P, KC, panel], mmdt, name="wt")
                for kc2 in range(KC // 2):
                    nc.sync.dma_start(
                        out=wt0[:, 2 * kc2 : 2 * kc2 + 2, :], in_=wb_d[kc2, :, 0, :]
                    )
                    for i in range(n_passes):
                        nc.sync.dma_start(
                            out=xs[i][:, 2 * kc2, :], in_=xt_rs[i][:, 2 * kc2, :]
                        )
                        nc.sync.dma_start(
                            out=xs[i][:, 2 * kc2 + 1, :],
                            in_=xt_rs[i][:, 2 * kc2 + 1, :],
                        )
                pss = [
                    pspool.tile([P, panel], dt.float32, name="ps") for _ in range(MT)
                ]
                for kc in range(KC):
                    for mt in range(MT):
                        for ipass in range(n_passes):
                            nc.tensor.matmul(
                                pss[mt][:],
                                lhsT=xs[ipass][:, kc, mt * P : (mt + 1) * P],
                                rhs=wt0[:, kc, :],
                                start=(kc == 0 and ipass == 0),
                                stop=(kc == KC - 1 and ipass == n_passes - 1),
                                skip_group_check=True,
                            )
                for mt in range(MT):
                    evict(pss[mt], mt, 0)

                # panels 1..NPAN-1: mt-outer (single psum in flight)
                for ip in range(1, NPAN):
                    wt = wpool.tile([P, KC, panel], mmdt, name="wt")
                    for kc2 in range(KC // 2):
                        nc.sync.dma_start(
                            out=wt[:, 2 * kc2 : 2 * kc2 + 2, :],
                            in_=wb_d[kc2, :, ip, :],
                        )
                    for mt in range(MT):
                        ps = pspool.tile([P, panel], dt.float32, name="ps")
                        for kc in range(KC):
                            for ipass in range(n_passes):
                                nc.tensor.matmul(
                                    ps[:],
                                    lhsT=xs[ipass][:, kc, mt * P : (mt + 1) * P],
                                    rhs=wt[:, kc, :],
                                    start=(kc == 0 and ipass == 0),
                                    stop=(kc == KC - 1 and ipass == n_passes - 1),
                                )
                        evict(ps, mt, ip)
    nc.compile()
    return nc


def _build_hybrid(G, repeat=1):
    """Hybrid fp8-DoubleRow + fp16 xstat kernel.

    K is split: first K8 = G*256 contraction rows go through fp8e4m3
    DoubleRow matmuls (2 k per PE cell per cycle, ~1.9x the fp16 rate);
    the remaining K16 = 4096 - K8 rows go through fp16 matmuls.  Both
    accumulate into the same PSUM bank per (mt, panel) stream.

    Per-core inputs (pre-packed on host):
      x8  [G, P, 2*M_CORE]          e4m3   x^T pair-tiles (k, k+128 within group)
      x16 [KC16, P, M_CORE]         fp16   x^T tiles for the fp16 part
      w8  [G, P, NPAN, 2*panel]     e4m3   binarized w panels, pair-packed
      w16 [KC16//2, P, NPAN, 2*panel] fp16 binarized w panels (kc-pair packed)
    Output: out [M_CORE, N_FULL] fp32.
    """
    import contextlib

    dt = mybir.dt
    panel = 512
    NPAN = N_FULL // panel
    MT = M_CORE // P
    K8 = G * 256
    KC16 = (K_DIM - K8) // P
    assert KC16 % 2 == 0
    # debug bisect flags (timing experiments only; break correctness)
    dbg_noxreload = bool(os.environ.get("BL_NOXRELOAD"))
    dbg_noevict = bool(os.environ.get("BL_NOEVICT"))
    dbg_nowdma = bool(os.environ.get("BL_NOWDMA"))

    nc = bacc.Bacc("TRN2")
    x8_d = nc.dram_tensor("x8", [G, P, 2 * M_CORE], dt.float8e4, kind="ExternalInput")
    x16_d = nc.dram_tensor("x16", [KC16, P, M_CORE], dt.float16, kind="ExternalInput")
    w8_d = nc.dram_tensor(
        "w8", [G, P, NPAN, 2 * panel], dt.float8e4, kind="ExternalInput"
    )
    w16_d = nc.dram_tensor(
        "w16", [KC16 // 2, P, NPAN, 2 * panel], dt.float16, kind="ExternalInput"
    )
    out_d = nc.dram_tensor("out", [M_CORE, N_FULL], dt.float32, kind="ExternalOutput")

    with TileContext(nc) as tc:
        with (
            tc.tile_pool(name="xpool", bufs=2) as xpool,
            tc.tile_pool(name="w8pool", bufs=2) as w8pool,
            tc.tile_pool(name="w16pool", bufs=2) as w16pool,
            tc.tile_pool(name="opool", bufs=3) as opool,
            tc.tile_pool(name="pspool", bufs=8, space="PSUM") as pspool,
        ):
            x8_r = x8_d.rearrange("g p (two m) -> g p two m", two=2)

            loop_cm = (
                tc.For_i(
                    0,
                    repeat,
                    1,
                    hint_engines=(
                        mybir.EngineType.PE,
                        mybir.EngineType.SP,
                        mybir.EngineType.DVE,
                    ),
                    name="rep",
                )
                if repeat > 1
                else contextlib.nullcontext()
            )

            def evict(ps, mt, ip):
                if dbg_noevict:
                    return
                ot = opool.tile([P, panel], dt.float32, name="ot")
                nc.vector.tensor_copy(ot[:], ps[:])
                nc.sync.dma_start(
                    out=out_d[mt * P : (mt + 1) * P, ip * panel : (ip + 1) * panel],
                    in_=ot[:],
                )

            def mm_stream(ps, x8t, x16t, w8t, w16t, mt, g_lo=0, g_hi=None,
                          kc_lo=0, kc_hi=None, first=True, last=True):
                g_hi = G if g_hi is None else g_hi
                kc_hi = KC16 if kc_hi is None else kc_hi
                for g in range(g_lo, g_hi):
                    nc.tensor.matmul(
                        ps[:],
                        lhsT=x8t[:, g, :, mt * P : (mt + 1) * P],
                        rhs=w8t[:, g, :, :],
                        start=(g == g_lo and first),
                        stop=False,
                        perf_mode=mybir.MatmulPerfMode.DoubleRow,
                        skip_group_check=True,
                    )
                for kc in range(kc_lo, kc_hi):
                    nc.tensor.matmul(
                        ps[:],
                        lhsT=x16t[:, kc, mt * P : (mt + 1) * P],
                        rhs=w16t[:, kc, :],
                        start=(g_hi == g_lo and kc == kc_lo and first),
                        stop=(kc == kc_hi - 1 and last),
                        skip_group_check=True,
                    )

            w8_r = w8_d.rearrange("g p np (two n) -> g p np two n", two=2)
            w16_r = w16_d.rearrange("kc2 p np (two n) -> kc2 p np two n", two=2)

            if dbg_noxreload:
                x8t_pre = xpool.tile([P, G, 2, M_CORE], dt.float8e4, name="x8t")
                x16t_pre = xpool.tile([P, KC16, M_CORE], dt.float16, name="x16t")
                for g in range(G):
                    nc.sync.dma_start(out=x8t_pre[:, g, :, :], in_=x8_r[g])
                for kc in range(KC16):
                    nc.sync.dma_start(out=x16t_pre[:, kc, :], in_=x16_d[kc])

            with loop_cm:
                if dbg_noxreload:
                    x8t, x16t = x8t_pre, x16t_pre
                else:
                    x8t = xpool.tile([P, G, 2, M_CORE], dt.float8e4, name="x8t")
                    x16t = xpool.tile([P, KC16, M_CORE], dt.float16, name="x16t")
                # ---- panel 0: two mt-outer k-sections; section A's slabs DMA
                # first so its compute starts while section B's slabs stream in.
                w8t0 = w8pool.tile([P, G, 2, panel], dt.float8e4, name="w8t")
                w16t0 = w16pool.tile([P, KC16, panel], dt.float16, name="w16t")
                GA = (G + 1) // 2
                KA2 = (KC16 // 2) // 2
                KA = 2 * KA2
                for g in range(GA):
                    nc.sync.dma_start(out=w8t0[:, g, :, :], in_=w8_r[g, :, 0])
                    if not dbg_noxreload:
                        nc.sync.dma_start(out=x8t[:, g, :, :], in_=x8_r[g])
                for kc2 in range(KA2):
                    nc.sync.dma_start(
                        out=w16t0[:, 2 * kc2 : 2 * kc2 + 2, :], in_=w16_r[kc2, :, 0]
                    )
                    if not dbg_noxreload:
                        nc.sync.dma_start(out=x16t[:, 2 * kc2, :], in_=x16_d[2 * kc2])
                        nc.sync.dma_start(
                            out=x16t[:, 2 * kc2 + 1, :], in_=x16_d[2 * kc2 + 1]
                        )
                for g in range(GA, G):
                    nc.sync.dma_start(out=w8t0[:, g, :, :], in_=w8_r[g, :, 0])
                    if not dbg_noxreload:
                        nc.sync.dma_start(out=x8t[:, g, :, :], in_=x8_r[g])
                for kc2 in range(KA2, KC16 // 2):
                    nc.sync.dma_start(
                        out=w16t0[:, 2 * kc2 : 2 * kc2 + 2, :], in_=w16_r[kc2, :, 0]
                    )
                    if not dbg_noxreload:
                        nc.sync.dma_start(out=x16t[:, 2 * kc2, :], in_=x16_d[2 * kc2])
                        nc.sync.dma_start(
                            out=x16t[:, 2 * kc2 + 1, :], in_=x16_d[2 * kc2 + 1]
                        )
                pss = [
                    pspool.tile([P, panel], dt.float32, name="ps") for _ in range(MT)
                ]
                for mt in range(MT):
                    mm_stream(pss[mt], x8t, x16t, w8t0, w16t0, mt,
                              g_hi=GA, kc_hi=KA, last=False)
                for mt in range(MT):
                    mm_stream(pss[mt], x8t, x16t, w8t0, w16t0, mt,
                              g_lo=GA, kc_lo=KA, first=False)
                    evict(pss[mt], mt, 0)

                # ---- panels 1..NPAN-1: mt-outer, double-buffered w tiles
                for ip in range(1, NPAN):
                    if dbg_nowdma:
                        w8t, w16t = w8t0, w16t0
                    else:
                        w8t = w8pool.tile([P, G, 2, panel], dt.float8e4, name="w8t")
                        w16t = w16pool.tile([P, KC16, panel], dt.float16, name="w16t")
                        for g in range(G):
                            nc.sync.dma_start(out=w8t[:, g, :, :], in_=w8_r[g, :, ip])
                        for kc2 in range(KC16 // 2):
                            nc.sync.dma_start(
                                out=w16t[:, 2 * kc2 : 2 * kc2 + 2, :],
                                in_=w16_r[kc2, :, ip],
                            )
                    for mt in range(MT):
                        ps = pspool.tile([P, panel], dt.float32, name="ps")
                        mm_stream(ps, x8t, x16t, w8t, w16t, mt)
                        evict(ps, mt, ip)
    nc.compile()
    return nc


def _build_wstat_f32r(bchunk=512, repeat=1):
    """Variant B (f32r): stationary = w column tiles, moving = resident x^T,
    out transposed [N_FULL, M_CORE].

    Per-core inputs: xt [K, M_CORE] fp32, wb [K, N_FULL] fp32.
    Output: out_t [N_FULL, M_CORE] fp32 (host transposes back).
    """
    dt = mybir.dt
    KC = K_DIM // P
    JT = N_FULL // P  # 32 stationary column tiles of w
    BC = M_CORE // bchunk  # moving chunks of x

    nc = bacc.Bacc("TRN2")
    xt_d = nc.dram_tensor("xt", [K_DIM, M_CORE], dt.float32r, kind="ExternalInput")
    wb_d = nc.dram_tensor("wb", [K_DIM, N_FULL], dt.float32r, kind="ExternalInput")
    out_d = nc.dram_tensor("out", [N_FULL, M_CORE], dt.float32, kind="ExternalOutput")

    with TileContext(nc) as tc:
        with (
            tc.tile_pool(name="xpool", bufs=1) as xpool,
            tc.tile_pool(name="wpool", bufs=3) as wpool,
            tc.tile_pool(name="opool", bufs=3) as opool,
            tc.tile_pool(name="pspool", bufs=4, space="PSUM") as pspool,
        ):
            xt_r = xt_d.rearrange("(kc p) m -> p kc m", p=P)
            xtile = xpool.tile([P, KC, M_CORE], dt.float32r, name="xres")
            for half in range(2):  # split load so compute starts early
                h = KC // 2
                nc.sync.dma_start(
                    out=xtile[:, half * h : (half + 1) * h, :],
                    in_=xt_r[:, half * h : (half + 1) * h, :],
                )

            wb_r = wb_d.rearrange("(kc p) n -> p kc n", p=P)

            import contextlib

            loop_cm = (
                tc.For_i(
                    0,
                    repeat,
                    1,
                    hint_engines=(
                        mybir.EngineType.PE,
                        mybir.EngineType.SP,
                        mybir.EngineType.DVE,
                    ),
                    name="rep",
                )
                if repeat > 1
                else contextlib.nullcontext()
            )
            with loop_cm:
                _body_wstat(nc, tc, wpool, opool, pspool, wb_r, xtile, out_d, KC, JT, BC, bchunk)
    nc.compile()
    return nc


def _body_wstat(nc, tc, wpool, opool, pspool, wb_r, xtile, out_d, KC, JT, BC, bchunk):
    dt = mybir.dt
    for jt in range(JT):
        wt = wpool.tile([P, KC, P], dt.float32r, name="wt")
        for kc in range(KC):
            nc.sync.dma_start(
                out=wt[:, kc, :],
                in_=wb_r[:, kc, jt * P : (jt + 1) * P],
            )
        pss = []
        for bc in range(BC):
            ps = pspool.tile([P, bchunk], dt.float32, name="ps")
            pss.append(ps)
        for kc in range(KC):
            for bc in range(BC):
                nc.tensor.matmul(
                    pss[bc][:],
                    lhsT=wt[:, kc, :],
                    rhs=xtile[:, kc, bc * bchunk : (bc + 1) * bchunk],
                    start=(kc == 0),
                    stop=(kc == KC - 1),
                    skip_group_check=True,
                )
        for bc in range(BC):
            ot = opool.tile([P, bchunk], dt.float32, name="ot")
            nc.vector.tensor_copy(ot[:], pss[bc][:])
            nc.sync.dma_start(
                out=out_d[jt * P : (jt + 1) * P, bc * bchunk : (bc + 1) * bchunk],
                in_=ot[:],
            )


def get_nc(variant=None, repeat=1):
    variant = variant or VARIANT
    key = (variant, repeat)
    if key not in _NC_CACHE:
        if variant == "f16x1":
            _NC_CACHE[key] = _build_xstat(1, mybir.dt.float16, 512, repeat)
        elif variant == "f16x2":
            _NC_CACHE[key] = _build_xstat(2, mybir.dt.float16, 256, repeat)
        elif variant == "bf16x2":
            _NC_CACHE[key] = _build_xstat(2, mybir.dt.bfloat16, 256, repeat)
        elif variant == "f32r":
            _NC_CACHE[key] = _build_wstat_f32r(512, repeat)
        elif variant.startswith("hyb"):
            _NC_CACHE[key] = _build_hybrid(int(variant[3:]), repeat)
        else:
            raise ValueError(f"unknown variant {variant}")
    return _NC_CACHE[key]


def _prep_hybrid(x, w, G):
    """Host prep for the hybrid variant: binarize w, split K, quantize/pack."""
    panel = 512
    NPAN = N_FULL // panel
    K8 = G * 256
    KC16 = (K_DIM - K8) // P
    f8 = ml_dtypes.float8_e4m3

    x = np.ascontiguousarray(x, dtype=np.float32)
    wb = np.where(np.asarray(w) >= 0, np.float32(1.0), np.float32(-1.0))
    xt = np.ascontiguousarray(x.T)  # [K, B]

    # x fp8 part: [K8, B] -> [G, 2, P, B] -> [G, P, 2, B]
    x8 = xt[:K8].astype(f8).reshape(G, 2, P, B_FULL).transpose(0, 2, 1, 3)
    x8 = np.ascontiguousarray(x8)  # [G, P, 2, B]
    # x fp16 part: [K16, B] -> [KC16, P, B]
    x16 = np.ascontiguousarray(xt[K8:].astype(np.float16).reshape(KC16, P, B_FULL))

    # w fp8 part: [K8, N] -> (G, 2, P, N) -> (G, P, 2, NPAN, panel) -> [G, P, NPAN, 2*panel]
    w8 = (
        wb[:K8]
        .astype(f8)
        .reshape(G, 2, P, NPAN, panel)
        .transpose(0, 2, 3, 1, 4)
        .reshape(G, P, NPAN, 2 * panel)
    )
    w8 = np.ascontiguousarray(w8)
    # w fp16 part: baseline kc-pair packing
    w16 = np.ascontiguousarray(
        wb[K8:]
        .astype(np.float16)
        .reshape(KC16 // 2, 2, P, NPAN, panel)
        .transpose(0, 2, 3, 1, 4)
        .reshape(KC16 // 2, P, NPAN, 2 * panel)
    )

    in_maps = []
    for c in range(NCORES):
        sl = slice(c * M_CORE, (c + 1) * M_CORE)
        in_maps.append(
            {
                "x8": np.ascontiguousarray(x8[:, :, :, sl]).reshape(G, P, 2 * M_CORE),
                "x16": np.ascontiguousarray(x16[:, :, sl]),
                "w8": w8,
                "w16": w16,
            }
        )
    return in_maps


def prep_in_maps(x, w, variant=None):
    """Host-side prep: binarize w, transpose/cast/split x, build per-core maps."""
    variant = variant or VARIANT
    if variant.startswith("hyb"):
        return _prep_hybrid(x, w, int(variant[3:]))
    x = np.ascontiguousarray(x, dtype=np.float32)
    wb = np.where(np.asarray(w) >= 0, np.float32(1.0), np.float32(-1.0))
    xt = np.ascontiguousarray(x.T)  # [K, B]

    if variant == "f32r":
        in_maps = []
        for c in range(NCORES):
            sl = slice(c * M_CORE, (c + 1) * M_CORE)
            in_maps.append({"xt": np.ascontiguousarray(xt[:, sl]), "wb": wb})
        return in_maps

    npdt = {"f16x1": np.float16, "f16x2": np.float16, "bf16x2": ml_dtypes.bfloat16}[
        variant
    ]
    n_passes = 1 if variant == "f16x1" else 2
    panel = 512 if variant == "f16x1" else 256
    KC, NPAN = K_DIM // P, N_FULL // panel
    xt_hi = xt.astype(npdt)
    wb16 = np.ascontiguousarray(
        wb.astype(npdt)
        .reshape(KC // 2, 2, P, NPAN, panel)
        .transpose(0, 2, 3, 1, 4)
        .reshape(KC // 2, P, NPAN, 2 * panel)
    )
    if n_passes == 2:
        xt_lo = (xt - xt_hi.astype(np.float32)).astype(npdt)

    in_maps = []
    for c in range(NCORES):
        sl = slice(c * M_CORE, (c + 1) * M_CORE)
        m = {"xt_hi": np.ascontiguousarray(xt_hi[:, sl]), "wb": wb16}
        if n_passes == 2:
            m["xt_lo"] = np.ascontiguousarray(xt_lo[:, sl])
        in_maps.append(m)
    return in_maps


def gather_out(results, variant=None):
    variant = variant or VARIANT
    if variant == "f32r":
        return np.concatenate(
            [np.asarray(results[c]["out"]).T for c in range(NCORES)], axis=0
        )
    return np.concatenate([np.asarray(results[c]["out"]) for c in range(NCORES)], axis=0)


def kernel(x, w):
    """Full inputs in, full output out.  x [8192, 4096] f32, w [4096, 4096] f32."""
    assert x.shape == (B_FULL, K_DIM) and w.shape == (K_DIM, N_FULL)
    nc = get_nc()
    in_maps = prep_in_maps(x, w)
    res = run_bass_kernel_spmd(nc, in_maps, core_ids=list(range(NCORES)))
    out = gather_out(res.results)
    return np.ascontiguousarray(out, dtype=np.float32)

